# revision 33
# baseline (speedup 1.0000x reference)
# Trainium2 Bass kernel for AttentionPooling (segment softmax-pool).
#
# Math: reference's per-slot max subtraction cancels in the softmax, so
#   w[t,k] = exp(s_t) / D_k,  D_k = sum_{t in slot_k} exp(s_t)
#   out[k,:] = sum_{t in slot_k} exp(s_t) * proj[t,:] / D_k
# (b2 shifts every score equally so it cancels too and is dropped.)
# With A[t,k] = in_slot(t,k) * exp(s_t), both numerator and D come from one
# accumulated PE matmul per 128-row chunk:  [num | D] += A^T @ [proj | 1].
#
# The score MLP needs proj with H on partitions; instead of shipping a second
# bf16 copy (baseline: 2x HBM), the h-major copy is shipped in fp8e4m3 and the
# W1 matmul runs in DoubleRow perf mode (256-row contraction in one pass at
# 0.5 cyc/col). Scores only feed exp(); fp8 keeps rel err ~9e-3, well under
# the 2e-2 gate. HBM per core: 8.4MB bf16 t-major + 4.2MB fp8 h-major.
#
# The slot masks A are built as e*(start<=t) - e*(end<=t): one 2K-wide int16
# tensor_scalar compare (DVE 2x mode) + one bf16 subtract, split across
# DVE and GpSimd by chunk to balance engine load.
#
# Sharding: data-parallel over B; core i handles batches 2i, 2i+1.

import numpy as np
import ml_dtypes

import concourse.bacc as bacc
import concourse.tile as tile
import concourse.mybir as mybir
import concourse.bass as bass
from concourse.bass_utils import run_bass_kernel_spmd

B, T, H, K = 16, 8192, 256, 128
HQ = 64
NCORES = 8
BPC = B // NCORES          # batches per core
CH = 128                   # rows per chunk
NCH = T // CH              # 64 chunks per batch
GRP = 8                    # chunks per DMA job
SUB = 4                    # chunks per W1-matmul/tanh subgroup

F32 = mybir.dt.float32
BF16 = mybir.dt.bfloat16
I16 = mybir.dt.int16
FP8 = mybir.dt.float8e4

# chunk c's mask-gen runs on Pool (gpsimd) when c % POOL_MOD < POOL_TAKE
POOL_MOD = 4
POOL_TAKE = 1


def make_jobs():
    """Job list in PE/accumulation order: (b, c0, n, preloaded).

    Batch BPC-1 ends with chunks 56-63 PRELOADED (g data DMA'd at kernel
    start) processed before a tapered final streamed group (chunks 48-55 as
    4/2/1/1), so the post-stream tail chain is a single 1-chunk matmul.
    """
    jobs = []
    last_b = BPC - 1
    c_pre = NCH - GRP            # 56: preloaded group (first in PE order)
    c_str = NCH - 2 * GRP        # 48: tapered streamed group (last)
    for b in range(BPC):
        NG = NCH // GRP
        if b == last_b:
            jobs.append((b, c_pre, GRP, True))
            jobs.append((b, c_str - GRP, GRP, True))
            for G in range(NG - 3):
                jobs.append((b, G * GRP, GRP, False))
            jobs += [
                (b, c_str, 4, False),
                (b, c_str + 4, 2, False),
                (b, c_str + 6, 1, False),
                (b, c_str + 7, 1, False),
            ]
        else:
            for G in range(NG):
                jobs.append((b, G * GRP, GRP, False))
    return jobs


def build_program():
    nc = bacc.Bacc(None, target_bir_lowering=False, debug=False)

    # t-major bf16 proj with the ones-column baked in at h=H (feeds the
    # denominator row of the seg matmul without a per-job memset), host-tiled
    # [b, G, p, g, h] so each partition reads one contiguous run per job
    NG = NCH // GRP
    proj = nc.dram_tensor("proj", [BPC, NG, CH, GRP, H + 1], BF16, kind="ExternalInput")
    # h-major fp8 proj for scores: [b, p, half, t] = proj[b, t, 128*half+p]
    projq = nc.dram_tensor("projq", [BPC, CH, 2, T], FP8, kind="ExternalInput")
    # starts and ends packed: [2, b, k], int16 (dead slots zeroed)
    bounds = nc.dram_tensor("bounds", [2, BPC, K], I16, kind="ExternalInput")
    # W1 halves for DoubleRow: [p, half, hq] = W1[128*half+p, hq]
    wpack = nc.dram_tensor("wpack", [CH, 2, HQ], FP8, kind="ExternalInput")
    w2t = nc.dram_tensor("w2t", [HQ], BF16, kind="ExternalInput")
    b1 = nc.dram_tensor("b1", [HQ], F32, kind="ExternalInput")
    # raw [num | D] per slot, f16; host divides num/D (exact softmax normalize)
    out = nc.dram_tensor("out", [BPC, K, H + 1], mybir.dt.float16, kind="ExternalOutput")

    with tile.TileContext(nc) as tc:
        with (
            tc.tile_pool(name="const", bufs=1) as const,
            tc.tile_pool(name="projg", bufs=8) as projp,
            tc.tile_pool(name="projtg", bufs=8) as ptp,
            tc.tile_pool(name="htanh", bufs=6) as htp,
            tc.tile_pool(name="amask", bufs=20) as apool,
            tc.tile_pool(name="eall", bufs=2) as epool,
            tc.tile_pool(name="outs", bufs=2) as outp,
            tc.tile_pool(name="misc", bufs=2) as miscp,
            tc.tile_pool(name="psH", bufs=3, space="PSUM") as psH,
            tc.tile_pool(name="psS", bufs=2, space="PSUM") as psS,
            tc.tile_pool(name="psSeg", bufs=2, space="PSUM") as psSeg,
        ):
            jobs = make_jobs()
            last_issued = {}
            first_issued = {}
            for b_, c0_, n_, _pre in jobs:
                last_issued[b_] = c0_ + n_ - 1
                if b_ not in first_issued:
                    first_issued[b_] = c0_
            e_alls = [
                epool.tile([CH, NCH], F32, tag="eall", name=f"e_all{b}")
                for b in range(BPC)
            ]
            segs = [
                psSeg.tile([K, H + 1], F32, tag="seg", name=f"seg{b}")
                for b in range(BPC)
            ]

            def dma_pt(b, c0, n):
                pt_tile = ptp.tile([CH, 2, GRP * CH], FP8, tag="pt")
                nc.sync.dma_start(
                    out=pt_tile[:, :, 0 : n * CH],
                    in_=bass.AP(
                        projq,
                        b * CH * 2 * T + c0 * CH,
                        [[2 * T, CH], [T, 2], [1, n * CH]],
                    ),
                )
                return pt_tile

            # kick off the first score slab before the constant loads
            pt_tiles = {0: dma_pt(*jobs[0][:3])}

            # ---- constants ----
            # tcol[p, c] = p + 128*c  (t coordinate of row p in chunk c)
            tcol = const.tile([CH, NCH], F32)
            nc.gpsimd.iota(
                tcol[:],
                pattern=[[CH, NCH]],
                base=0,
                channel_multiplier=1,
                allow_small_or_imprecise_dtypes=True,
            )

            # wp is needed by the very first W1 matmul: Act HWDGE queue (one
            # small HWDGE slot at the head). The rest go through Pool SWDGE so
            # they never contend with the SP proj stream on HWDGE.
            wp = const.tile([CH, 2, HQ], FP8)
            nc.scalar.dma_start(
                out=wp[:],
                in_=bass.AP(wpack, 0, [[2 * HQ, CH], [HQ, 2], [1, HQ]]),
            )
            w2_sb = const.tile([HQ, 1], BF16)
            nc.gpsimd.dma_start(out=w2_sb[:], in_=bass.AP(w2t, 0, [[1, HQ], [1, 1]]))
            b1_sb = const.tile([HQ, 1], F32)
            nc.gpsimd.dma_start(out=b1_sb[:], in_=bass.AP(b1, 0, [[1, HQ], [1, 1]]))

            # boundaries broadcast down all 128 partitions: [p, se, b, k]
            bnd = const.tile([CH, 2, BPC, K], I16)
            nc.gpsimd.dma_start(
                out=bnd[:],
                in_=bass.AP(bounds, 0, [[0, CH], [BPC * K, 2], [K, BPC], [1, K]]),
            )

            def scores(b, c0, n, pt_tile, pt_off=0):
                e_all = e_alls[b]
                s_ps = psS.tile([CH, GRP], F32, tag="sps")
                for s0 in range(0, n, SUB):
                    ns = min(SUB, n - s0)
                    po = pt_off + s0
                    hps = psH.tile([HQ, SUB * CH], F32, tag="hps")
                    nc.tensor.matmul(
                        hps[:, 0 : ns * CH],
                        wp[:],
                        pt_tile[:, :, po * CH : (po + ns) * CH],
                        start=True,
                        stop=True,
                        perf_mode=mybir.MatmulPerfMode.DoubleRow,
                    )
                    hts = htp.tile([HQ, SUB * CH], BF16, tag="hts")
                    nc.scalar.activation(
                        out=hts[:, 0 : ns * CH],
                        in_=hps[:, 0 : ns * CH],
                        func=mybir.ActivationFunctionType.Tanh,
                        bias=b1_sb[:],
                        scale=1.0,
                    )
                    for j in range(ns):
                        nc.tensor.matmul(
                            s_ps[:, s0 + j : s0 + j + 1],
                            hts[:, j * CH : (j + 1) * CH],
                            w2_sb[:],
                            start=True,
                            stop=True,
                        )
                nc.scalar.activation(
                    out=e_all[:, c0 : c0 + n],
                    in_=s_ps[:, 0:n],
                    func=mybir.ActivationFunctionType.Exp,
                )

            def agen(b, c0, n, hold=False):
                e_all = e_alls[b]
                a2s = []
                for g in range(n):
                    c = c0 + g
                    eng = nc.gpsimd if (c % POOL_MOD < POOL_TAKE) else nc.vector
                    cmp = apool.tile([CH, 2, K], BF16, tag="a1", bufs=10)
                    # tail-job masks are built at kernel start and consumed at
                    # the very end: a dedicated ring so the streaming masks
                    # never wait on them for buffer reuse
                    a2 = apool.tile(
                        [CH, K], BF16, tag="a2h" if hold else "a2",
                        bufs=16 if hold else 24,
                    )
                    # cmp[t, 0, k] = (start_k <= t)*E_t ; cmp[t, 1, k] = (end_k <= t)*E_t
                    eng.tensor_scalar(
                        out=cmp[:],
                        in0=bnd[:, :, b, :],
                        scalar1=tcol[:, c : c + 1],
                        scalar2=e_all[:, c : c + 1],
                        op0=mybir.AluOpType.is_le,
                        op1=mybir.AluOpType.mult,
                    )
                    # a2[t,k] = E_t * ((start_k<=t) - (end_k<=t)) = E_t * in_slot
                    eng.tensor_tensor(
                        out=a2[:],
                        in0=cmp[:, 0, :],
                        in1=cmp[:, 1, :],
                        op=mybir.AluOpType.subtract,
                    )
                    a2s.append(a2)
                return a2s

            def dma_g(b, c0, n, tag="g", bufs=None):
                HP = H + 1
                g_tile = projp.tile(
                    [CH, GRP, HP], BF16, tag=tag, name="g_tile",
                    **({"bufs": bufs} if bufs else {}),
                )
                G, g0 = c0 // GRP, c0 % GRP
                nc.sync.dma_start(
                    out=g_tile[:, 0:n, :],
                    in_=bass.AP(
                        proj,
                        (b * (NCH // GRP) + G) * CH * GRP * HP + g0 * HP,
                        [[GRP * HP, CH], [HP, n], [1, HP]],
                    ),
                )
                return g_tile

            def seg_mms(b, c0, n, a2s, g_tile):
                seg = segs[b]
                for g in range(n):
                    c = c0 + g
                    nc.tensor.matmul(
                        seg[:],
                        a2s[g][:],
                        g_tile[:, g, :],
                        start=(c == first_issued[b]),
                        stop=(c == last_issued[b]),
                    )

            def epilogue(b):
                # copy PSUM [num | D] to SBUF f16; the out DMA is deferred to
                # program end so it can't head-of-line block input DMAs on the
                # SP sequencer (it dispatches in order).
                seg = segs[b]
                ot = outp.tile([K, H + 1], mybir.dt.float16, name=f"ot{b}")
                nc.scalar.copy(out=ot[:], in_=seg[:])
                return ot

            def out_dma(b, ot):
                nc.sync.dma_start(
                    out=bass.AP(out, b * K * (H + 1), [[H + 1, K], [1, H + 1]]),
                    in_=ot[:],
                )

            # scores are emitted in a different order than seg: job 0 first,
            # then the 5 tail jobs (preloaded + tapered — their E-weights,
            # masks AND the preloaded job's seg data are all resident at the
            # START, so the post-stream tail is one 1-chunk matmul chain),
            # then the middle jobs one per seg iteration.
            last_jx = {}
            for jx, (b_, c0_, n_, _pre) in enumerate(jobs):
                last_jx[b_] = jx
            pre_set = [jx for jx, j in enumerate(jobs) if j[3]]
            taper_set = [
                jx for jx, j in enumerate(jobs) if not j[3] and j[2] < GRP
            ]
            lead_set = pre_set + taper_set
            last_b = BPC - 1
            c_tap = min(jobs[jx][1] for jx in taper_set)
            score_seq = [0] + [
                jx for jx in range(1, len(jobs)) if jx not in lead_set
            ]
            a2_map = {}
            g_pre = {}
            # lead: scores+masks for the preloaded job and (as one fused pass)
            # the tapered tail jobs, plus two stream jobs of score headroom
            for sx in pre_set:
                pt_tiles[sx] = dma_pt(*jobs[sx][:3])
            pt_tap = dma_pt(last_b, c_tap, GRP)  # one fetch covers all taper jobs
            scores(*jobs[0][:3], pt_tiles.pop(0))
            for sx in pre_set:
                scores(*jobs[sx][:3], pt_tiles.pop(sx))
                a2_map[sx] = agen(*jobs[sx][:3], hold=True)
                g_pre[sx] = dma_g(*jobs[sx][:3], tag="gpre", bufs=2)
            scores(last_b, c_tap, GRP, pt_tap)
            for sx in taper_set:
                a2_map[sx] = agen(*jobs[sx][:3], hold=True)
            nlead = 3
            for sx in score_seq[1:nlead]:
                pt_tiles[sx] = dma_pt(*jobs[sx][:3])
                scores(*jobs[sx][:3], pt_tiles.pop(sx))
            # batch b's epilogue copy is issued 2 jobs after its seg chain
            # stops (so the Act queue never waits on it) and its out DMA one
            # job later; the final batch drains at program end.
            copy_at = {last_jx[b_] + 4: b_ for b_ in range(BPC)}
            dma_at = {last_jx[b_] + 6: b_ for b_ in range(BPC)}
            # masks are generated two jobs ahead of their seg matmuls so the
            # DVE/Pool queues never gate the PE at the stream tail
            for aj in (0, 1):
                if aj not in a2_map:
                    a2_map[aj] = agen(*jobs[aj][:3])
            nxt = nlead
            ots = {}
            for jx, (b, c0, n, pre) in enumerate(jobs):
                if nxt < len(score_seq):
                    sx = score_seq[nxt]
                    nxt += 1
                    pt_tiles[sx] = dma_pt(*jobs[sx][:3])
                    scores(*jobs[sx][:3], pt_tiles.pop(sx))
                aj = jx + 2
                if aj < len(jobs) and aj not in a2_map:
                    a2_map[aj] = agen(*jobs[aj][:3])
                if jx not in a2_map:
                    a2_map[jx] = agen(b, c0, n)
                g_tile = g_pre.pop(jx) if pre else dma_g(b, c0, n)
                seg_mms(b, c0, n, a2_map.pop(jx), g_tile)
                if jx in copy_at:
                    ots[copy_at[jx]] = epilogue(copy_at[jx])
                if jx in dma_at:
                    out_dma(dma_at[jx], ots[dma_at[jx]])
            for b_ in range(BPC):
                if b_ not in ots:
                    ots[b_] = epilogue(b_)
                if last_jx[b_] + 6 >= len(jobs):
                    out_dma(b_, ots[b_])

    nc.compile()
    return nc


_prog_cache = None
LAST_RESULTS = None


def _get_program():
    global _prog_cache
    if _prog_cache is None:
        _prog_cache = build_program()
    return _prog_cache


def kernel(**inputs):
    proj = np.asarray(inputs["projected"], dtype=np.float32)
    bnds = np.asarray(inputs["boundaries"])
    slot = np.asarray(inputs["slot_mask"])
    W1 = np.asarray(inputs["W1"], dtype=np.float32)
    b1 = np.ascontiguousarray(np.asarray(inputs["b1"], dtype=np.float32))
    W2 = np.asarray(inputs["W2"], dtype=np.float32).reshape(HQ)

    live = slot > 0
    starts = np.where(live, bnds[..., 0], 0).astype(np.int16)     # [B, K]
    ends = np.where(live, bnds[..., 1], 0).astype(np.int16)

    # h-major fp8 for scores: [B, p, half, T]
    projq = np.ascontiguousarray(
        proj.astype(ml_dtypes.float8_e4m3)
        .transpose(0, 2, 1)
        .reshape(B, 2, CH, T)
        .transpose(0, 2, 1, 3)
    )
    # [B, T, H+1] (ones col baked in) -> [B, G, p, g, h]
    proj_bf = np.empty((B, T, H + 1), dtype=ml_dtypes.bfloat16)
    proj_bf[:, :, :H] = proj
    proj_bf[:, :, H] = 1.0
    proj_bf = np.ascontiguousarray(
        proj_bf.reshape(B, NCH // GRP, GRP, CH, H + 1).transpose(0, 1, 3, 2, 4)
    )

    # W1 packed for DoubleRow: [p, half, hq]
    wpack = np.ascontiguousarray(
        W1.reshape(2, CH, HQ).transpose(1, 0, 2)
    ).astype(ml_dtypes.float8_e4m3)
    w2t = W2.astype(ml_dtypes.bfloat16)

    nc = _get_program()
    in_maps = []
    for i in range(NCORES):
        lo, hi = i * BPC, (i + 1) * BPC
        in_maps.append(
            {
                "proj": proj_bf[lo:hi],
                "projq": projq[lo:hi],
                "bounds": np.ascontiguousarray(
                    np.stack([starts[lo:hi], ends[lo:hi]])
                ),
                "wpack": wpack,
                "w2t": w2t,
                "b1": b1,
            }
        )

    res = run_bass_kernel_spmd(nc, in_maps, core_ids=list(range(NCORES)))
    global LAST_RESULTS
    LAST_RESULTS = res
    raw = np.concatenate(
        [np.asarray(r["out"]) for r in res.results], axis=0
    ).astype(np.float32)                                           # [B, K, H+1]
    num, den = raw[..., :H], raw[..., H:]
    return num / np.where(den > 0, den, 1.0)


# revision 34
# speedup vs baseline: 1.0060x; 1.0060x over previous
# Trainium2 Bass kernel for AttentionPooling (segment softmax-pool).
#
# Math: reference's per-slot max subtraction cancels in the softmax, so
#   w[t,k] = exp(s_t) / D_k,  D_k = sum_{t in slot_k} exp(s_t)
#   out[k,:] = sum_{t in slot_k} exp(s_t) * proj[t,:] / D_k
# (b2 shifts every score equally so it cancels too and is dropped.)
# With A[t,k] = in_slot(t,k) * exp(s_t), both numerator and D come from one
# accumulated PE matmul per 128-row chunk:  [num | D] += A^T @ [proj | 1].
#
# The score MLP needs proj with H on partitions; instead of shipping a second
# bf16 copy (baseline: 2x HBM), the h-major copy is shipped in fp8e4m3 and the
# W1 matmul runs in DoubleRow perf mode (256-row contraction in one pass at
# 0.5 cyc/col). Scores only feed exp(); fp8 keeps rel err ~9e-3, well under
# the 2e-2 gate. HBM per core: 8.4MB bf16 t-major + 4.2MB fp8 h-major.
#
# The slot masks A are built as e*(start<=t) - e*(end<=t): one 2K-wide int16
# tensor_scalar compare (DVE 2x mode) + one bf16 subtract, split across
# DVE and GpSimd by chunk to balance engine load.
#
# Sharding: data-parallel over B; core i handles batches 2i, 2i+1.

import numpy as np
import ml_dtypes

import concourse.bacc as bacc
import concourse.tile as tile
import concourse.mybir as mybir
import concourse.bass as bass
from concourse.bass_utils import run_bass_kernel_spmd

B, T, H, K = 16, 8192, 256, 128
HQ = 64
NCORES = 8
BPC = B // NCORES          # batches per core
CH = 128                   # rows per chunk
NCH = T // CH              # 64 chunks per batch
GRP = 8                    # chunks per DMA job
SUB = 4                    # chunks per W1-matmul/tanh subgroup

F32 = mybir.dt.float32
BF16 = mybir.dt.bfloat16
I16 = mybir.dt.int16
FP8 = mybir.dt.float8e4

# chunk c's mask-gen runs on Pool (gpsimd) when c % POOL_MOD < POOL_TAKE
POOL_MOD = 4
POOL_TAKE = 1


def make_jobs():
    """Job list in PE/accumulation order: (b, c0, n, preloaded).

    Batch BPC-1 ends with chunks 56-63 PRELOADED (g data DMA'd at kernel
    start) processed before a tapered final streamed group (chunks 48-55 as
    4/2/1/1), so the post-stream tail chain is a single 1-chunk matmul.
    """
    jobs = []
    last_b = BPC - 1
    c_pre = NCH - GRP            # 56: preloaded group (first in PE order)
    c_str = NCH - 2 * GRP        # 48: tapered streamed group (last)
    for b in range(BPC):
        NG = NCH // GRP
        if b == last_b:
            jobs.append((b, c_pre, GRP, True))
            for G in range(NG - 2):
                jobs.append((b, G * GRP, GRP, False))
            jobs += [
                (b, c_str, 4, False),
                (b, c_str + 4, 2, False),
                (b, c_str + 6, 1, False),
                (b, c_str + 7, 1, False),
            ]
        else:
            for G in range(NG):
                jobs.append((b, G * GRP, GRP, False))
    return jobs


def build_program():
    nc = bacc.Bacc(None, target_bir_lowering=False, debug=False)

    # t-major bf16 proj with the ones-column baked in at h=H (feeds the
    # denominator row of the seg matmul without a per-job memset), host-tiled
    # [b, G, p, g, h] so each partition reads one contiguous run per job
    NG = NCH // GRP
    proj = nc.dram_tensor("proj", [BPC, NG, CH, GRP, H + 1], BF16, kind="ExternalInput")
    # h-major fp8 proj for scores: [b, p, half, t] = proj[b, t, 128*half+p]
    projq = nc.dram_tensor("projq", [BPC, CH, 2, T], FP8, kind="ExternalInput")
    # starts and ends packed: [2, b, k], int16 (dead slots zeroed)
    bounds = nc.dram_tensor("bounds", [2, BPC, K], I16, kind="ExternalInput")
    # W1 halves for DoubleRow: [p, half, hq] = W1[128*half+p, hq]
    wpack = nc.dram_tensor("wpack", [CH, 2, HQ], FP8, kind="ExternalInput")
    w2t = nc.dram_tensor("w2t", [HQ], BF16, kind="ExternalInput")
    b1 = nc.dram_tensor("b1", [HQ], F32, kind="ExternalInput")
    # raw [num | D] per slot, f16; host divides num/D (exact softmax normalize)
    out = nc.dram_tensor("out", [BPC, K, H + 1], mybir.dt.float16, kind="ExternalOutput")

    with tile.TileContext(nc) as tc:
        with (
            tc.tile_pool(name="const", bufs=1) as const,
            tc.tile_pool(name="projg", bufs=8) as projp,
            tc.tile_pool(name="projtg", bufs=8) as ptp,
            tc.tile_pool(name="htanh", bufs=6) as htp,
            tc.tile_pool(name="amask", bufs=20) as apool,
            tc.tile_pool(name="eall", bufs=2) as epool,
            tc.tile_pool(name="outs", bufs=2) as outp,
            tc.tile_pool(name="misc", bufs=2) as miscp,
            tc.tile_pool(name="psH", bufs=3, space="PSUM") as psH,
            tc.tile_pool(name="psS", bufs=2, space="PSUM") as psS,
            tc.tile_pool(name="psSeg", bufs=2, space="PSUM") as psSeg,
        ):
            jobs = make_jobs()
            last_issued = {}
            first_issued = {}
            for b_, c0_, n_, _pre in jobs:
                last_issued[b_] = c0_ + n_ - 1
                if b_ not in first_issued:
                    first_issued[b_] = c0_
            e_alls = [
                epool.tile([CH, NCH], F32, tag="eall", name=f"e_all{b}")
                for b in range(BPC)
            ]
            segs = [
                psSeg.tile([K, H + 1], F32, tag="seg", name=f"seg{b}")
                for b in range(BPC)
            ]

            def dma_pt(b, c0, n):
                pt_tile = ptp.tile([CH, 2, GRP * CH], FP8, tag="pt")
                nc.sync.dma_start(
                    out=pt_tile[:, :, 0 : n * CH],
                    in_=bass.AP(
                        projq,
                        b * CH * 2 * T + c0 * CH,
                        [[2 * T, CH], [T, 2], [1, n * CH]],
                    ),
                )
                return pt_tile

            # kick off the first score slab before the constant loads
            pt_tiles = {0: dma_pt(*jobs[0][:3])}

            # ---- constants ----
            # tcol[p, c] = p + 128*c  (t coordinate of row p in chunk c)
            tcol = const.tile([CH, NCH], F32)
            nc.gpsimd.iota(
                tcol[:],
                pattern=[[CH, NCH]],
                base=0,
                channel_multiplier=1,
                allow_small_or_imprecise_dtypes=True,
            )

            # wp is needed by the very first W1 matmul: Act HWDGE queue (one
            # small HWDGE slot at the head). The rest go through Pool SWDGE so
            # they never contend with the SP proj stream on HWDGE.
            wp = const.tile([CH, 2, HQ], FP8)
            nc.scalar.dma_start(
                out=wp[:],
                in_=bass.AP(wpack, 0, [[2 * HQ, CH], [HQ, 2], [1, HQ]]),
            )
            w2_sb = const.tile([HQ, 1], BF16)
            nc.gpsimd.dma_start(out=w2_sb[:], in_=bass.AP(w2t, 0, [[1, HQ], [1, 1]]))
            b1_sb = const.tile([HQ, 1], F32)
            nc.gpsimd.dma_start(out=b1_sb[:], in_=bass.AP(b1, 0, [[1, HQ], [1, 1]]))

            # boundaries broadcast down all 128 partitions: [p, se, b, k]
            bnd = const.tile([CH, 2, BPC, K], I16)
            nc.gpsimd.dma_start(
                out=bnd[:],
                in_=bass.AP(bounds, 0, [[0, CH], [BPC * K, 2], [K, BPC], [1, K]]),
            )

            def scores(b, c0, n, pt_tile, pt_off=0):
                e_all = e_alls[b]
                s_ps = psS.tile([CH, GRP], F32, tag="sps")
                for s0 in range(0, n, SUB):
                    ns = min(SUB, n - s0)
                    po = pt_off + s0
                    hps = psH.tile([HQ, SUB * CH], F32, tag="hps")
                    nc.tensor.matmul(
                        hps[:, 0 : ns * CH],
                        wp[:],
                        pt_tile[:, :, po * CH : (po + ns) * CH],
                        start=True,
                        stop=True,
                        perf_mode=mybir.MatmulPerfMode.DoubleRow,
                    )
                    hts = htp.tile([HQ, SUB * CH], BF16, tag="hts")
                    nc.scalar.activation(
                        out=hts[:, 0 : ns * CH],
                        in_=hps[:, 0 : ns * CH],
                        func=mybir.ActivationFunctionType.Tanh,
                        bias=b1_sb[:],
                        scale=1.0,
                    )
                    for j in range(ns):
                        nc.tensor.matmul(
                            s_ps[:, s0 + j : s0 + j + 1],
                            hts[:, j * CH : (j + 1) * CH],
                            w2_sb[:],
                            start=True,
                            stop=True,
                        )
                nc.scalar.activation(
                    out=e_all[:, c0 : c0 + n],
                    in_=s_ps[:, 0:n],
                    func=mybir.ActivationFunctionType.Exp,
                )

            def agen(b, c0, n, hold=False):
                e_all = e_alls[b]
                a2s = []
                for g in range(n):
                    c = c0 + g
                    eng = nc.gpsimd if (c % POOL_MOD < POOL_TAKE) else nc.vector
                    cmp = apool.tile([CH, 2, K], BF16, tag="a1", bufs=10)
                    # tail-job masks are built at kernel start and consumed at
                    # the very end: a dedicated ring so the streaming masks
                    # never wait on them for buffer reuse
                    a2 = apool.tile(
                        [CH, K], BF16, tag="a2h" if hold else "a2",
                        bufs=16 if hold else 24,
                    )
                    # cmp[t, 0, k] = (start_k <= t)*E_t ; cmp[t, 1, k] = (end_k <= t)*E_t
                    eng.tensor_scalar(
                        out=cmp[:],
                        in0=bnd[:, :, b, :],
                        scalar1=tcol[:, c : c + 1],
                        scalar2=e_all[:, c : c + 1],
                        op0=mybir.AluOpType.is_le,
                        op1=mybir.AluOpType.mult,
                    )
                    # a2[t,k] = E_t * ((start_k<=t) - (end_k<=t)) = E_t * in_slot
                    eng.tensor_tensor(
                        out=a2[:],
                        in0=cmp[:, 0, :],
                        in1=cmp[:, 1, :],
                        op=mybir.AluOpType.subtract,
                    )
                    a2s.append(a2)
                return a2s

            def dma_g(b, c0, n, tag="g", bufs=None):
                HP = H + 1
                g_tile = projp.tile(
                    [CH, GRP, HP], BF16, tag=tag, name="g_tile",
                    **({"bufs": bufs} if bufs else {}),
                )
                G, g0 = c0 // GRP, c0 % GRP
                nc.sync.dma_start(
                    out=g_tile[:, 0:n, :],
                    in_=bass.AP(
                        proj,
                        (b * (NCH // GRP) + G) * CH * GRP * HP + g0 * HP,
                        [[GRP * HP, CH], [HP, n], [1, HP]],
                    ),
                )
                return g_tile

            def seg_mms(b, c0, n, a2s, g_tile):
                seg = segs[b]
                for g in range(n):
                    c = c0 + g
                    nc.tensor.matmul(
                        seg[:],
                        a2s[g][:],
                        g_tile[:, g, :],
                        start=(c == first_issued[b]),
                        stop=(c == last_issued[b]),
                    )

            def epilogue(b):
                # copy PSUM [num | D] to SBUF f16; the out DMA is deferred to
                # program end so it can't head-of-line block input DMAs on the
                # SP sequencer (it dispatches in order).
                seg = segs[b]
                ot = outp.tile([K, H + 1], mybir.dt.float16, name=f"ot{b}")
                nc.scalar.copy(out=ot[:], in_=seg[:])
                return ot

            def out_dma(b, ot):
                nc.sync.dma_start(
                    out=bass.AP(out, b * K * (H + 1), [[H + 1, K], [1, H + 1]]),
                    in_=ot[:],
                )

            # scores are emitted in a different order than seg: job 0 first,
            # then the 5 tail jobs (preloaded + tapered — their E-weights,
            # masks AND the preloaded job's seg data are all resident at the
            # START, so the post-stream tail is one 1-chunk matmul chain),
            # then the middle jobs one per seg iteration.
            last_jx = {}
            for jx, (b_, c0_, n_, _pre) in enumerate(jobs):
                last_jx[b_] = jx
            pre_set = [jx for jx, j in enumerate(jobs) if j[3]]
            taper_set = [
                jx for jx, j in enumerate(jobs) if not j[3] and j[2] < GRP
            ]
            lead_set = pre_set + taper_set
            last_b = BPC - 1
            c_tap = min(jobs[jx][1] for jx in taper_set)
            score_seq = [0] + [
                jx for jx in range(1, len(jobs)) if jx not in lead_set
            ]
            a2_map = {}
            g_pre = {}
            # lead: scores+masks for the preloaded job and (as one fused pass)
            # the tapered tail jobs, plus two stream jobs of score headroom
            for sx in pre_set:
                pt_tiles[sx] = dma_pt(*jobs[sx][:3])
            pt_tap = dma_pt(last_b, c_tap, GRP)  # one fetch covers all taper jobs
            scores(*jobs[0][:3], pt_tiles.pop(0))
            for sx in pre_set:
                scores(*jobs[sx][:3], pt_tiles.pop(sx))
                a2_map[sx] = agen(*jobs[sx][:3], hold=True)
                g_pre[sx] = dma_g(*jobs[sx][:3], tag="gpre", bufs=2)
            scores(last_b, c_tap, GRP, pt_tap)
            for sx in taper_set:
                a2_map[sx] = agen(*jobs[sx][:3], hold=True)
            nlead = 3
            for sx in score_seq[1:nlead]:
                pt_tiles[sx] = dma_pt(*jobs[sx][:3])
                scores(*jobs[sx][:3], pt_tiles.pop(sx))
            # batch b's epilogue copy is issued 2 jobs after its seg chain
            # stops (so the Act queue never waits on it) and its out DMA one
            # job later; the final batch drains at program end.
            copy_at = {last_jx[b_] + 4: b_ for b_ in range(BPC)}
            dma_at = {last_jx[b_] + 6: b_ for b_ in range(BPC)}
            # masks are generated two jobs ahead of their seg matmuls so the
            # DVE/Pool queues never gate the PE at the stream tail
            for aj in (0, 1):
                if aj not in a2_map:
                    a2_map[aj] = agen(*jobs[aj][:3])
            nxt = nlead
            ots = {}
            for jx, (b, c0, n, pre) in enumerate(jobs):
                if nxt < len(score_seq):
                    sx = score_seq[nxt]
                    nxt += 1
                    pt_tiles[sx] = dma_pt(*jobs[sx][:3])
                    scores(*jobs[sx][:3], pt_tiles.pop(sx))
                aj = jx + 2
                if aj < len(jobs) and aj not in a2_map:
                    a2_map[aj] = agen(*jobs[aj][:3])
                if jx not in a2_map:
                    a2_map[jx] = agen(b, c0, n)
                g_tile = g_pre.pop(jx) if pre else dma_g(b, c0, n)
                seg_mms(b, c0, n, a2_map.pop(jx), g_tile)
                if jx in copy_at:
                    ots[copy_at[jx]] = epilogue(copy_at[jx])
                if jx in dma_at:
                    out_dma(dma_at[jx], ots[dma_at[jx]])
            for b_ in range(BPC):
                if b_ not in ots:
                    ots[b_] = epilogue(b_)
                if last_jx[b_] + 6 >= len(jobs):
                    out_dma(b_, ots[b_])

    nc.compile()
    return nc


_prog_cache = None
LAST_RESULTS = None


def _get_program():
    global _prog_cache
    if _prog_cache is None:
        _prog_cache = build_program()
    return _prog_cache


def kernel(**inputs):
    proj = np.asarray(inputs["projected"], dtype=np.float32)
    bnds = np.asarray(inputs["boundaries"])
    slot = np.asarray(inputs["slot_mask"])
    W1 = np.asarray(inputs["W1"], dtype=np.float32)
    b1 = np.ascontiguousarray(np.asarray(inputs["b1"], dtype=np.float32))
    W2 = np.asarray(inputs["W2"], dtype=np.float32).reshape(HQ)

    live = slot > 0
    starts = np.where(live, bnds[..., 0], 0).astype(np.int16)     # [B, K]
    ends = np.where(live, bnds[..., 1], 0).astype(np.int16)

    # h-major fp8 for scores: [B, p, half, T]
    projq = np.ascontiguousarray(
        proj.astype(ml_dtypes.float8_e4m3)
        .transpose(0, 2, 1)
        .reshape(B, 2, CH, T)
        .transpose(0, 2, 1, 3)
    )
    # [B, T, H+1] (ones col baked in) -> [B, G, p, g, h]
    proj_bf = np.empty((B, T, H + 1), dtype=ml_dtypes.bfloat16)
    proj_bf[:, :, :H] = proj
    proj_bf[:, :, H] = 1.0
    proj_bf = np.ascontiguousarray(
        proj_bf.reshape(B, NCH // GRP, GRP, CH, H + 1).transpose(0, 1, 3, 2, 4)
    )

    # W1 packed for DoubleRow: [p, half, hq]
    wpack = np.ascontiguousarray(
        W1.reshape(2, CH, HQ).transpose(1, 0, 2)
    ).astype(ml_dtypes.float8_e4m3)
    w2t = W2.astype(ml_dtypes.bfloat16)

    nc = _get_program()
    in_maps = []
    for i in range(NCORES):
        lo, hi = i * BPC, (i + 1) * BPC
        in_maps.append(
            {
                "proj": proj_bf[lo:hi],
                "projq": projq[lo:hi],
                "bounds": np.ascontiguousarray(
                    np.stack([starts[lo:hi], ends[lo:hi]])
                ),
                "wpack": wpack,
                "w2t": w2t,
                "b1": b1,
            }
        )

    res = run_bass_kernel_spmd(nc, in_maps, core_ids=list(range(NCORES)))
    global LAST_RESULTS
    LAST_RESULTS = res
    raw = np.concatenate(
        [np.asarray(r["out"]) for r in res.results], axis=0
    ).astype(np.float32)                                           # [B, K, H+1]
    num, den = raw[..., :H], raw[..., H:]
    return num / np.where(den > 0, den, 1.0)


# revision 35
# speedup vs baseline: 1.0066x; 1.0005x over previous
# Trainium2 Bass kernel for AttentionPooling (segment softmax-pool).
#
# Math: reference's per-slot max subtraction cancels in the softmax, so
#   w[t,k] = exp(s_t) / D_k,  D_k = sum_{t in slot_k} exp(s_t)
#   out[k,:] = sum_{t in slot_k} exp(s_t) * proj[t,:] / D_k
# (b2 shifts every score equally so it cancels too and is dropped.)
# With A[t,k] = in_slot(t,k) * exp(s_t), both numerator and D come from one
# accumulated PE matmul per 128-row chunk:  [num | D] += A^T @ [proj | 1].
#
# The score MLP needs proj with H on partitions; instead of shipping a second
# bf16 copy (baseline: 2x HBM), the h-major copy is shipped in fp8e4m3 and the
# W1 matmul runs in DoubleRow perf mode (256-row contraction in one pass at
# 0.5 cyc/col). Scores only feed exp(); fp8 keeps rel err ~9e-3, well under
# the 2e-2 gate. HBM per core: 8.4MB bf16 t-major + 4.2MB fp8 h-major.
#
# The slot masks A are built as e*(start<=t) - e*(end<=t): one 2K-wide int16
# tensor_scalar compare (DVE 2x mode) + one bf16 subtract, split across
# DVE and GpSimd by chunk to balance engine load.
#
# Sharding: data-parallel over B; core i handles batches 2i, 2i+1.

import numpy as np
import ml_dtypes

import concourse.bacc as bacc
import concourse.tile as tile
import concourse.mybir as mybir
import concourse.bass as bass
from concourse.bass_utils import run_bass_kernel_spmd

B, T, H, K = 16, 8192, 256, 128
HQ = 64
NCORES = 8
BPC = B // NCORES          # batches per core
CH = 128                   # rows per chunk
NCH = T // CH              # 64 chunks per batch
GRP = 8                    # chunks per DMA job
SUB = 4                    # chunks per W1-matmul/tanh subgroup

F32 = mybir.dt.float32
BF16 = mybir.dt.bfloat16
I16 = mybir.dt.int16
FP8 = mybir.dt.float8e4

# chunk c's mask-gen runs on Pool (gpsimd) when c % POOL_MOD < POOL_TAKE
POOL_MOD = 4
POOL_TAKE = 1


def make_jobs():
    """Job list in PE/accumulation order: (b, c0, n, preloaded).

    Batch BPC-1 ends with chunks 56-63 PRELOADED (g data DMA'd at kernel
    start) processed before a tapered final streamed group (chunks 48-55 as
    4/2/1/1), so the post-stream tail chain is a single 1-chunk matmul.
    """
    jobs = []
    last_b = BPC - 1
    c_pre = NCH - GRP            # 56: preloaded group (first in PE order)
    c_str = NCH - 2 * GRP        # 48: tapered streamed group (last)
    for b in range(BPC):
        NG = NCH // GRP
        if b == last_b:
            jobs.append((b, c_pre, GRP, True))
            for G in range(NG - 2):
                jobs.append((b, G * GRP, GRP, False))
            jobs += [
                (b, c_str, 4, False),
                (b, c_str + 4, 2, False),
                (b, c_str + 6, 1, False),
                (b, c_str + 7, 1, False),
            ]
        else:
            for G in range(NG):
                jobs.append((b, G * GRP, GRP, False))
    return jobs


def build_program():
    nc = bacc.Bacc(None, target_bir_lowering=False, debug=False)

    # t-major bf16 proj with the ones-column baked in at h=H (feeds the
    # denominator row of the seg matmul without a per-job memset), host-tiled
    # [b, G, p, g, h] so each partition reads one contiguous run per job
    NG = NCH // GRP
    proj = nc.dram_tensor("proj", [BPC, NG, CH, GRP, H + 1], BF16, kind="ExternalInput")
    # h-major fp8 proj for scores: [b, p, half, t] = proj[b, t, 128*half+p]
    projq = nc.dram_tensor("projq", [BPC, CH, 2, T], FP8, kind="ExternalInput")
    # starts and ends packed: [2, b, k], int16 (dead slots zeroed)
    bounds = nc.dram_tensor("bounds", [2, BPC, K], I16, kind="ExternalInput")
    # W1 halves for DoubleRow: [p, half, hq] = W1[128*half+p, hq]
    wpack = nc.dram_tensor("wpack", [CH, 2, HQ], FP8, kind="ExternalInput")
    w2t = nc.dram_tensor("w2t", [HQ], BF16, kind="ExternalInput")
    b1 = nc.dram_tensor("b1", [HQ], F32, kind="ExternalInput")
    # raw [num | D] per slot, f16; host divides num/D (exact softmax normalize)
    out = nc.dram_tensor("out", [BPC, K, H + 1], mybir.dt.float16, kind="ExternalOutput")

    with tile.TileContext(nc) as tc:
        with (
            tc.tile_pool(name="const", bufs=1) as const,
            tc.tile_pool(name="projg", bufs=8) as projp,
            tc.tile_pool(name="projtg", bufs=8) as ptp,
            tc.tile_pool(name="htanh", bufs=6) as htp,
            tc.tile_pool(name="amask", bufs=20) as apool,
            tc.tile_pool(name="eall", bufs=2) as epool,
            tc.tile_pool(name="outs", bufs=2) as outp,
            tc.tile_pool(name="misc", bufs=2) as miscp,
            tc.tile_pool(name="psH", bufs=3, space="PSUM") as psH,
            tc.tile_pool(name="psS", bufs=2, space="PSUM") as psS,
            tc.tile_pool(name="psSeg", bufs=2, space="PSUM") as psSeg,
        ):
            jobs = make_jobs()
            last_issued = {}
            first_issued = {}
            for b_, c0_, n_, _pre in jobs:
                last_issued[b_] = c0_ + n_ - 1
                if b_ not in first_issued:
                    first_issued[b_] = c0_
            e_alls = [
                epool.tile([CH, NCH], F32, tag="eall", name=f"e_all{b}")
                for b in range(BPC)
            ]
            segs = [
                psSeg.tile([K, H + 1], F32, tag="seg", name=f"seg{b}")
                for b in range(BPC)
            ]

            def dma_pt(b, c0, n):
                pt_tile = ptp.tile([CH, 2, GRP * CH], FP8, tag="pt")
                nc.sync.dma_start(
                    out=pt_tile[:, :, 0 : n * CH],
                    in_=bass.AP(
                        projq,
                        b * CH * 2 * T + c0 * CH,
                        [[2 * T, CH], [T, 2], [1, n * CH]],
                    ),
                )
                return pt_tile

            # kick off the first score slab before the constant loads
            pt_tiles = {0: dma_pt(*jobs[0][:3])}

            # ---- constants ----
            # tcol[p, c] = p + 128*c  (t coordinate of row p in chunk c)
            tcol = const.tile([CH, NCH], F32)
            nc.gpsimd.iota(
                tcol[:],
                pattern=[[CH, NCH]],
                base=0,
                channel_multiplier=1,
                allow_small_or_imprecise_dtypes=True,
            )

            # wp is needed by the very first W1 matmul: Act HWDGE queue (one
            # small HWDGE slot at the head). The rest go through Pool SWDGE so
            # they never contend with the SP proj stream on HWDGE.
            wp = const.tile([CH, 2, HQ], FP8)
            nc.scalar.dma_start(
                out=wp[:],
                in_=bass.AP(wpack, 0, [[2 * HQ, CH], [HQ, 2], [1, HQ]]),
            )
            w2_sb = const.tile([HQ, 1], BF16)
            nc.gpsimd.dma_start(out=w2_sb[:], in_=bass.AP(w2t, 0, [[1, HQ], [1, 1]]))
            b1_sb = const.tile([HQ, 1], F32)
            nc.gpsimd.dma_start(out=b1_sb[:], in_=bass.AP(b1, 0, [[1, HQ], [1, 1]]))

            # boundaries broadcast down all 128 partitions: [p, se, b, k]
            bnd = const.tile([CH, 2, BPC, K], I16)
            nc.gpsimd.dma_start(
                out=bnd[:],
                in_=bass.AP(bounds, 0, [[0, CH], [BPC * K, 2], [K, BPC], [1, K]]),
            )

            def scores(b, c0, n, pt_tile, pt_off=0):
                e_all = e_alls[b]
                s_ps = psS.tile([CH, GRP], F32, tag="sps")
                for s0 in range(0, n, SUB):
                    ns = min(SUB, n - s0)
                    po = pt_off + s0
                    hps = psH.tile([HQ, SUB * CH], F32, tag="hps")
                    nc.tensor.matmul(
                        hps[:, 0 : ns * CH],
                        wp[:],
                        pt_tile[:, :, po * CH : (po + ns) * CH],
                        start=True,
                        stop=True,
                        perf_mode=mybir.MatmulPerfMode.DoubleRow,
                    )
                    hts = htp.tile([HQ, SUB * CH], BF16, tag="hts")
                    nc.scalar.activation(
                        out=hts[:, 0 : ns * CH],
                        in_=hps[:, 0 : ns * CH],
                        func=mybir.ActivationFunctionType.Tanh,
                        bias=b1_sb[:],
                        scale=1.0,
                    )
                    for j in range(ns):
                        nc.tensor.matmul(
                            s_ps[:, s0 + j : s0 + j + 1],
                            hts[:, j * CH : (j + 1) * CH],
                            w2_sb[:],
                            start=True,
                            stop=True,
                        )
                nc.scalar.activation(
                    out=e_all[:, c0 : c0 + n],
                    in_=s_ps[:, 0:n],
                    func=mybir.ActivationFunctionType.Exp,
                )

            def agen(b, c0, n, hold=False):
                e_all = e_alls[b]
                a2s = []
                for g in range(n):
                    c = c0 + g
                    eng = nc.gpsimd if (c % POOL_MOD < POOL_TAKE) else nc.vector
                    cmp = apool.tile([CH, 2, K], BF16, tag="a1", bufs=10)
                    # tail-job masks are built at kernel start and consumed at
                    # the very end: a dedicated ring so the streaming masks
                    # never wait on them for buffer reuse
                    a2 = apool.tile(
                        [CH, K], BF16, tag="a2h" if hold else "a2",
                        bufs=16 if hold else 32,
                    )
                    # cmp[t, 0, k] = (start_k <= t)*E_t ; cmp[t, 1, k] = (end_k <= t)*E_t
                    eng.tensor_scalar(
                        out=cmp[:],
                        in0=bnd[:, :, b, :],
                        scalar1=tcol[:, c : c + 1],
                        scalar2=e_all[:, c : c + 1],
                        op0=mybir.AluOpType.is_le,
                        op1=mybir.AluOpType.mult,
                    )
                    # a2[t,k] = E_t * ((start_k<=t) - (end_k<=t)) = E_t * in_slot
                    eng.tensor_tensor(
                        out=a2[:],
                        in0=cmp[:, 0, :],
                        in1=cmp[:, 1, :],
                        op=mybir.AluOpType.subtract,
                    )
                    a2s.append(a2)
                return a2s

            def dma_g(b, c0, n, tag="g", bufs=None):
                HP = H + 1
                g_tile = projp.tile(
                    [CH, GRP, HP], BF16, tag=tag, name="g_tile",
                    **({"bufs": bufs} if bufs else {}),
                )
                G, g0 = c0 // GRP, c0 % GRP
                nc.sync.dma_start(
                    out=g_tile[:, 0:n, :],
                    in_=bass.AP(
                        proj,
                        (b * (NCH // GRP) + G) * CH * GRP * HP + g0 * HP,
                        [[GRP * HP, CH], [HP, n], [1, HP]],
                    ),
                )
                return g_tile

            def seg_mms(b, c0, n, a2s, g_tile):
                seg = segs[b]
                for g in range(n):
                    c = c0 + g
                    nc.tensor.matmul(
                        seg[:],
                        a2s[g][:],
                        g_tile[:, g, :],
                        start=(c == first_issued[b]),
                        stop=(c == last_issued[b]),
                    )

            def epilogue(b):
                # copy PSUM [num | D] to SBUF f16; the out DMA is deferred to
                # program end so it can't head-of-line block input DMAs on the
                # SP sequencer (it dispatches in order).
                seg = segs[b]
                ot = outp.tile([K, H + 1], mybir.dt.float16, name=f"ot{b}")
                nc.scalar.copy(out=ot[:], in_=seg[:])
                return ot

            def out_dma(b, ot):
                nc.sync.dma_start(
                    out=bass.AP(out, b * K * (H + 1), [[H + 1, K], [1, H + 1]]),
                    in_=ot[:],
                )

            # scores are emitted in a different order than seg: job 0 first,
            # then the 5 tail jobs (preloaded + tapered — their E-weights,
            # masks AND the preloaded job's seg data are all resident at the
            # START, so the post-stream tail is one 1-chunk matmul chain),
            # then the middle jobs one per seg iteration.
            last_jx = {}
            for jx, (b_, c0_, n_, _pre) in enumerate(jobs):
                last_jx[b_] = jx
            pre_set = [jx for jx, j in enumerate(jobs) if j[3]]
            taper_set = [
                jx for jx, j in enumerate(jobs) if not j[3] and j[2] < GRP
            ]
            lead_set = pre_set + taper_set
            last_b = BPC - 1
            c_tap = min(jobs[jx][1] for jx in taper_set)
            score_seq = [0] + [
                jx for jx in range(1, len(jobs)) if jx not in lead_set
            ]
            a2_map = {}
            g_pre = {}
            # lead: scores+masks for the preloaded job and (as one fused pass)
            # the tapered tail jobs, plus two stream jobs of score headroom
            for sx in pre_set:
                pt_tiles[sx] = dma_pt(*jobs[sx][:3])
            pt_tap = dma_pt(last_b, c_tap, GRP)  # one fetch covers all taper jobs
            scores(*jobs[0][:3], pt_tiles.pop(0))
            for sx in pre_set:
                scores(*jobs[sx][:3], pt_tiles.pop(sx))
                a2_map[sx] = agen(*jobs[sx][:3], hold=True)
                g_pre[sx] = dma_g(*jobs[sx][:3], tag="gpre", bufs=2)
            scores(last_b, c_tap, GRP, pt_tap)
            for sx in taper_set:
                a2_map[sx] = agen(*jobs[sx][:3], hold=True)
            nlead = 3
            for sx in score_seq[1:nlead]:
                pt_tiles[sx] = dma_pt(*jobs[sx][:3])
                scores(*jobs[sx][:3], pt_tiles.pop(sx))
            # batch b's epilogue copy is issued 2 jobs after its seg chain
            # stops (so the Act queue never waits on it) and its out DMA one
            # job later; the final batch drains at program end.
            copy_at = {last_jx[b_] + 4: b_ for b_ in range(BPC)}
            dma_at = {last_jx[b_] + 6: b_ for b_ in range(BPC)}
            # masks are generated two jobs ahead of their seg matmuls so the
            # DVE/Pool queues never gate the PE at the stream tail
            for aj in (0, 1, 2):
                if aj not in a2_map:
                    a2_map[aj] = agen(*jobs[aj][:3])
            nxt = nlead
            ots = {}
            for jx, (b, c0, n, pre) in enumerate(jobs):
                if nxt < len(score_seq):
                    sx = score_seq[nxt]
                    nxt += 1
                    pt_tiles[sx] = dma_pt(*jobs[sx][:3])
                    scores(*jobs[sx][:3], pt_tiles.pop(sx))
                aj = jx + 3
                if aj < len(jobs) and aj not in a2_map:
                    a2_map[aj] = agen(*jobs[aj][:3])
                if jx not in a2_map:
                    a2_map[jx] = agen(b, c0, n)
                g_tile = g_pre.pop(jx) if pre else dma_g(b, c0, n)
                seg_mms(b, c0, n, a2_map.pop(jx), g_tile)
                if jx in copy_at:
                    ots[copy_at[jx]] = epilogue(copy_at[jx])
                if jx in dma_at:
                    out_dma(dma_at[jx], ots[dma_at[jx]])
            for b_ in range(BPC):
                if b_ not in ots:
                    ots[b_] = epilogue(b_)
                if last_jx[b_] + 6 >= len(jobs):
                    out_dma(b_, ots[b_])

    nc.compile()
    return nc


_prog_cache = None
LAST_RESULTS = None


def _get_program():
    global _prog_cache
    if _prog_cache is None:
        _prog_cache = build_program()
    return _prog_cache


def kernel(**inputs):
    proj = np.asarray(inputs["projected"], dtype=np.float32)
    bnds = np.asarray(inputs["boundaries"])
    slot = np.asarray(inputs["slot_mask"])
    W1 = np.asarray(inputs["W1"], dtype=np.float32)
    b1 = np.ascontiguousarray(np.asarray(inputs["b1"], dtype=np.float32))
    W2 = np.asarray(inputs["W2"], dtype=np.float32).reshape(HQ)

    live = slot > 0
    starts = np.where(live, bnds[..., 0], 0).astype(np.int16)     # [B, K]
    ends = np.where(live, bnds[..., 1], 0).astype(np.int16)

    # h-major fp8 for scores: [B, p, half, T]
    projq = np.ascontiguousarray(
        proj.astype(ml_dtypes.float8_e4m3)
        .transpose(0, 2, 1)
        .reshape(B, 2, CH, T)
        .transpose(0, 2, 1, 3)
    )
    # [B, T, H+1] (ones col baked in) -> [B, G, p, g, h]
    proj_bf = np.empty((B, T, H + 1), dtype=ml_dtypes.bfloat16)
    proj_bf[:, :, :H] = proj
    proj_bf[:, :, H] = 1.0
    proj_bf = np.ascontiguousarray(
        proj_bf.reshape(B, NCH // GRP, GRP, CH, H + 1).transpose(0, 1, 3, 2, 4)
    )

    # W1 packed for DoubleRow: [p, half, hq]
    wpack = np.ascontiguousarray(
        W1.reshape(2, CH, HQ).transpose(1, 0, 2)
    ).astype(ml_dtypes.float8_e4m3)
    w2t = W2.astype(ml_dtypes.bfloat16)

    nc = _get_program()
    in_maps = []
    for i in range(NCORES):
        lo, hi = i * BPC, (i + 1) * BPC
        in_maps.append(
            {
                "proj": proj_bf[lo:hi],
                "projq": projq[lo:hi],
                "bounds": np.ascontiguousarray(
                    np.stack([starts[lo:hi], ends[lo:hi]])
                ),
                "wpack": wpack,
                "w2t": w2t,
                "b1": b1,
            }
        )

    res = run_bass_kernel_spmd(nc, in_maps, core_ids=list(range(NCORES)))
    global LAST_RESULTS
    LAST_RESULTS = res
    raw = np.concatenate(
        [np.asarray(r["out"]) for r in res.results], axis=0
    ).astype(np.float32)                                           # [B, K, H+1]
    num, den = raw[..., :H], raw[..., H:]
    return num / np.where(den > 0, den, 1.0)


# revision 36
# speedup vs baseline: 1.0283x; 1.0216x over previous
# Trainium2 Bass kernel for AttentionPooling (segment softmax-pool).
#
# Math: reference's per-slot max subtraction cancels in the softmax, so
#   w[t,k] = exp(s_t) / D_k,  D_k = sum_{t in slot_k} exp(s_t)
#   out[k,:] = sum_{t in slot_k} exp(s_t) * proj[t,:] / D_k
# (b2 shifts every score equally so it cancels too and is dropped.)
# With A[t,k] = in_slot(t,k) * exp(s_t), both numerator and D come from one
# accumulated PE matmul per 128-row chunk:  [num | D] += A^T @ [proj | 1].
#
# The score MLP needs proj with H on partitions; instead of shipping a second
# bf16 copy (baseline: 2x HBM), the h-major copy is shipped in fp8e4m3 and the
# W1 matmul runs in DoubleRow perf mode (256-row contraction in one pass at
# 0.5 cyc/col). Scores only feed exp(); fp8 keeps rel err ~9e-3, well under
# the 2e-2 gate. HBM per core: 8.4MB bf16 t-major + 4.2MB fp8 h-major.
#
# The slot masks A are built as e*(start<=t) - e*(end<=t): one 2K-wide int16
# tensor_scalar compare (DVE 2x mode) + one bf16 subtract, split across
# DVE and GpSimd by chunk to balance engine load.
#
# Sharding: data-parallel over B; core i handles batches 2i, 2i+1.

import numpy as np
import ml_dtypes

import concourse.bacc as bacc
import concourse.tile as tile
import concourse.mybir as mybir
import concourse.bass as bass
from concourse.bass_utils import run_bass_kernel_spmd

B, T, H, K = 16, 8192, 256, 128
HQ = 64
NCORES = 8
BPC = B // NCORES          # batches per core
CH = 128                   # rows per chunk
NCH = T // CH              # 64 chunks per batch
GRP = 8                    # chunks per DMA job
SUB = 4                    # chunks per W1-matmul/tanh subgroup

F32 = mybir.dt.float32
BF16 = mybir.dt.bfloat16
I16 = mybir.dt.int16
FP8 = mybir.dt.float8e4

# chunk c's mask-gen runs on Pool (gpsimd) when c % POOL_MOD < POOL_TAKE
POOL_MOD = 5
POOL_TAKE = 1


def make_jobs():
    """Job list in PE/accumulation order: (b, c0, n, preloaded).

    Batch BPC-1 ends with chunks 56-63 PRELOADED (g data DMA'd at kernel
    start) processed before a tapered final streamed group (chunks 48-55 as
    4/2/1/1), so the post-stream tail chain is a single 1-chunk matmul.
    """
    jobs = []
    last_b = BPC - 1
    c_pre = NCH - GRP            # 56: preloaded group (first in PE order)
    c_str = NCH - 2 * GRP        # 48: tapered streamed group (last)
    for b in range(BPC):
        NG = NCH // GRP
        if b == last_b:
            jobs.append((b, c_pre, GRP, True))
            for G in range(NG - 2):
                jobs.append((b, G * GRP, GRP, False))
            jobs += [
                (b, c_str, 4, False),
                (b, c_str + 4, 2, False),
                (b, c_str + 6, 1, False),
                (b, c_str + 7, 1, False),
            ]
        else:
            for G in range(NG):
                jobs.append((b, G * GRP, GRP, False))
    return jobs


def build_program():
    nc = bacc.Bacc(None, target_bir_lowering=False, debug=False)

    # t-major bf16 proj with the ones-column baked in at h=H (feeds the
    # denominator row of the seg matmul without a per-job memset), host-tiled
    # [b, G, p, g, h] so each partition reads one contiguous run per job
    NG = NCH // GRP
    proj = nc.dram_tensor("proj", [BPC, NG, CH, GRP, H + 1], BF16, kind="ExternalInput")
    # h-major fp8 proj for scores: [b, p, half, t] = proj[b, t, 128*half+p]
    projq = nc.dram_tensor("projq", [BPC, CH, 2, T], FP8, kind="ExternalInput")
    # starts and ends packed: [2, b, k], int16 (dead slots zeroed)
    bounds = nc.dram_tensor("bounds", [2, BPC, K], I16, kind="ExternalInput")
    # W1 halves for DoubleRow: [p, half, hq] = W1[128*half+p, hq]
    wpack = nc.dram_tensor("wpack", [CH, 2, HQ], FP8, kind="ExternalInput")
    w2t = nc.dram_tensor("w2t", [HQ], BF16, kind="ExternalInput")
    b1 = nc.dram_tensor("b1", [HQ], F32, kind="ExternalInput")
    # raw [num | D] per slot, f16; host divides num/D (exact softmax normalize)
    out = nc.dram_tensor("out", [BPC, K, H + 1], mybir.dt.float16, kind="ExternalOutput")

    with tile.TileContext(nc) as tc:
        with (
            tc.tile_pool(name="const", bufs=1) as const,
            tc.tile_pool(name="projg", bufs=8) as projp,
            tc.tile_pool(name="projtg", bufs=8) as ptp,
            tc.tile_pool(name="htanh", bufs=6) as htp,
            tc.tile_pool(name="amask", bufs=20) as apool,
            tc.tile_pool(name="eall", bufs=2) as epool,
            tc.tile_pool(name="outs", bufs=2) as outp,
            tc.tile_pool(name="misc", bufs=2) as miscp,
            tc.tile_pool(name="psH", bufs=3, space="PSUM") as psH,
            tc.tile_pool(name="psS", bufs=2, space="PSUM") as psS,
            tc.tile_pool(name="psSeg", bufs=2, space="PSUM") as psSeg,
        ):
            jobs = make_jobs()
            last_issued = {}
            first_issued = {}
            for b_, c0_, n_, _pre in jobs:
                last_issued[b_] = c0_ + n_ - 1
                if b_ not in first_issued:
                    first_issued[b_] = c0_
            e_alls = [
                epool.tile([CH, NCH], F32, tag="eall", name=f"e_all{b}")
                for b in range(BPC)
            ]
            segs = [
                psSeg.tile([K, H + 1], F32, tag="seg", name=f"seg{b}")
                for b in range(BPC)
            ]

            def dma_pt(b, c0, n):
                pt_tile = ptp.tile([CH, 2, GRP * CH], FP8, tag="pt")
                nc.sync.dma_start(
                    out=pt_tile[:, :, 0 : n * CH],
                    in_=bass.AP(
                        projq,
                        b * CH * 2 * T + c0 * CH,
                        [[2 * T, CH], [T, 2], [1, n * CH]],
                    ),
                )
                return pt_tile

            # kick off the first score slab before the constant loads
            pt_tiles = {0: dma_pt(*jobs[0][:3])}

            # ---- constants ----
            # tcol[p, c] = p + 128*c  (t coordinate of row p in chunk c)
            tcol = const.tile([CH, NCH], F32)
            nc.gpsimd.iota(
                tcol[:],
                pattern=[[CH, NCH]],
                base=0,
                channel_multiplier=1,
                allow_small_or_imprecise_dtypes=True,
            )

            # wp is needed by the very first W1 matmul: Act HWDGE queue (one
            # small HWDGE slot at the head). The rest go through Pool SWDGE so
            # they never contend with the SP proj stream on HWDGE.
            wp = const.tile([CH, 2, HQ], FP8)
            nc.scalar.dma_start(
                out=wp[:],
                in_=bass.AP(wpack, 0, [[2 * HQ, CH], [HQ, 2], [1, HQ]]),
            )
            w2_sb = const.tile([HQ, 1], BF16)
            nc.gpsimd.dma_start(out=w2_sb[:], in_=bass.AP(w2t, 0, [[1, HQ], [1, 1]]))
            b1_sb = const.tile([HQ, 1], F32)
            nc.gpsimd.dma_start(out=b1_sb[:], in_=bass.AP(b1, 0, [[1, HQ], [1, 1]]))

            # boundaries broadcast down all 128 partitions: [p, se, b, k]
            bnd = const.tile([CH, 2, BPC, K], I16)
            nc.gpsimd.dma_start(
                out=bnd[:],
                in_=bass.AP(bounds, 0, [[0, CH], [BPC * K, 2], [K, BPC], [1, K]]),
            )

            def scores(b, c0, n, pt_tile, pt_off=0):
                e_all = e_alls[b]
                s_ps = psS.tile([CH, GRP], F32, tag="sps")
                for s0 in range(0, n, SUB):
                    ns = min(SUB, n - s0)
                    po = pt_off + s0
                    hps = psH.tile([HQ, SUB * CH], F32, tag="hps")
                    nc.tensor.matmul(
                        hps[:, 0 : ns * CH],
                        wp[:],
                        pt_tile[:, :, po * CH : (po + ns) * CH],
                        start=True,
                        stop=True,
                        perf_mode=mybir.MatmulPerfMode.DoubleRow,
                    )
                    hts = htp.tile([HQ, SUB * CH], BF16, tag="hts")
                    nc.scalar.activation(
                        out=hts[:, 0 : ns * CH],
                        in_=hps[:, 0 : ns * CH],
                        func=mybir.ActivationFunctionType.Tanh,
                        bias=b1_sb[:],
                        scale=1.0,
                    )
                    for j in range(ns):
                        nc.tensor.matmul(
                            s_ps[:, s0 + j : s0 + j + 1],
                            hts[:, j * CH : (j + 1) * CH],
                            w2_sb[:],
                            start=True,
                            stop=True,
                        )
                nc.scalar.activation(
                    out=e_all[:, c0 : c0 + n],
                    in_=s_ps[:, 0:n],
                    func=mybir.ActivationFunctionType.Exp,
                )

            def agen(b, c0, n, hold=False):
                e_all = e_alls[b]
                a2s = []
                for g in range(n):
                    c = c0 + g
                    eng = nc.gpsimd if (c % POOL_MOD < POOL_TAKE) else nc.vector
                    cmp = apool.tile([CH, 2, K], BF16, tag="a1", bufs=10)
                    # tail-job masks are built at kernel start and consumed at
                    # the very end: a dedicated ring so the streaming masks
                    # never wait on them for buffer reuse
                    a2 = apool.tile(
                        [CH, K], BF16, tag="a2h" if hold else "a2",
                        bufs=16 if hold else 32,
                    )
                    # cmp[t, 0, k] = (start_k <= t)*E_t ; cmp[t, 1, k] = (end_k <= t)*E_t
                    eng.tensor_scalar(
                        out=cmp[:],
                        in0=bnd[:, :, b, :],
                        scalar1=tcol[:, c : c + 1],
                        scalar2=e_all[:, c : c + 1],
                        op0=mybir.AluOpType.is_le,
                        op1=mybir.AluOpType.mult,
                    )
                    # a2[t,k] = E_t * ((start_k<=t) - (end_k<=t)) = E_t * in_slot
                    eng.tensor_tensor(
                        out=a2[:],
                        in0=cmp[:, 0, :],
                        in1=cmp[:, 1, :],
                        op=mybir.AluOpType.subtract,
                    )
                    a2s.append(a2)
                return a2s

            def dma_g(b, c0, n, tag="g", bufs=None):
                HP = H + 1
                g_tile = projp.tile(
                    [CH, GRP, HP], BF16, tag=tag, name="g_tile",
                    **({"bufs": bufs} if bufs else {}),
                )
                G, g0 = c0 // GRP, c0 % GRP
                nc.sync.dma_start(
                    out=g_tile[:, 0:n, :],
                    in_=bass.AP(
                        proj,
                        (b * (NCH // GRP) + G) * CH * GRP * HP + g0 * HP,
                        [[GRP * HP, CH], [HP, n], [1, HP]],
                    ),
                )
                return g_tile

            def seg_mms(b, c0, n, a2s, g_tile):
                seg = segs[b]
                for g in range(n):
                    c = c0 + g
                    nc.tensor.matmul(
                        seg[:],
                        a2s[g][:],
                        g_tile[:, g, :],
                        start=(c == first_issued[b]),
                        stop=(c == last_issued[b]),
                    )

            def epilogue(b):
                # copy PSUM [num | D] to SBUF f16; the out DMA is deferred to
                # program end so it can't head-of-line block input DMAs on the
                # SP sequencer (it dispatches in order).
                seg = segs[b]
                ot = outp.tile([K, H + 1], mybir.dt.float16, name=f"ot{b}")
                nc.scalar.copy(out=ot[:], in_=seg[:])
                return ot

            def out_dma(b, ot):
                nc.sync.dma_start(
                    out=bass.AP(out, b * K * (H + 1), [[H + 1, K], [1, H + 1]]),
                    in_=ot[:],
                )

            # scores are emitted in a different order than seg: job 0 first,
            # then the 5 tail jobs (preloaded + tapered — their E-weights,
            # masks AND the preloaded job's seg data are all resident at the
            # START, so the post-stream tail is one 1-chunk matmul chain),
            # then the middle jobs one per seg iteration.
            last_jx = {}
            for jx, (b_, c0_, n_, _pre) in enumerate(jobs):
                last_jx[b_] = jx
            pre_set = [jx for jx, j in enumerate(jobs) if j[3]]
            taper_set = [
                jx for jx, j in enumerate(jobs) if not j[3] and j[2] < GRP
            ]
            lead_set = pre_set + taper_set
            last_b = BPC - 1
            c_tap = min(jobs[jx][1] for jx in taper_set)
            score_seq = [0] + [
                jx for jx in range(1, len(jobs)) if jx not in lead_set
            ]
            a2_map = {}
            g_pre = {}
            # lead: scores+masks for the preloaded job and (as one fused pass)
            # the tapered tail jobs, plus two stream jobs of score headroom
            for sx in pre_set:
                pt_tiles[sx] = dma_pt(*jobs[sx][:3])
            pt_tap = dma_pt(last_b, c_tap, GRP)  # one fetch covers all taper jobs
            scores(*jobs[0][:3], pt_tiles.pop(0))
            for sx in pre_set:
                scores(*jobs[sx][:3], pt_tiles.pop(sx))
                a2_map[sx] = agen(*jobs[sx][:3], hold=True)
                g_pre[sx] = dma_g(*jobs[sx][:3], tag="gpre", bufs=2)
            scores(last_b, c_tap, GRP, pt_tap)
            for sx in taper_set:
                a2_map[sx] = agen(*jobs[sx][:3], hold=True)
            nlead = 3
            for sx in score_seq[1:nlead]:
                pt_tiles[sx] = dma_pt(*jobs[sx][:3])
                scores(*jobs[sx][:3], pt_tiles.pop(sx))
            # batch b's epilogue copy is issued 2 jobs after its seg chain
            # stops (so the Act queue never waits on it) and its out DMA one
            # job later; the final batch drains at program end.
            copy_at = {last_jx[b_] + 4: b_ for b_ in range(BPC)}
            dma_at = {last_jx[b_] + 6: b_ for b_ in range(BPC)}
            # masks are generated two jobs ahead of their seg matmuls so the
            # DVE/Pool queues never gate the PE at the stream tail
            for aj in (0, 1, 2):
                if aj not in a2_map:
                    a2_map[aj] = agen(*jobs[aj][:3])
            nxt = nlead
            ots = {}
            for jx, (b, c0, n, pre) in enumerate(jobs):
                if nxt < len(score_seq):
                    sx = score_seq[nxt]
                    nxt += 1
                    pt_tiles[sx] = dma_pt(*jobs[sx][:3])
                    scores(*jobs[sx][:3], pt_tiles.pop(sx))
                aj = jx + 3
                if aj < len(jobs) and aj not in a2_map:
                    a2_map[aj] = agen(*jobs[aj][:3])
                if jx not in a2_map:
                    a2_map[jx] = agen(b, c0, n)
                g_tile = g_pre.pop(jx) if pre else dma_g(b, c0, n)
                seg_mms(b, c0, n, a2_map.pop(jx), g_tile)
                if jx in copy_at:
                    ots[copy_at[jx]] = epilogue(copy_at[jx])
                if jx in dma_at:
                    out_dma(dma_at[jx], ots[dma_at[jx]])
            for b_ in range(BPC):
                if b_ not in ots:
                    ots[b_] = epilogue(b_)
                if last_jx[b_] + 6 >= len(jobs):
                    out_dma(b_, ots[b_])

    nc.compile()
    return nc


_prog_cache = None
LAST_RESULTS = None


def _get_program():
    global _prog_cache
    if _prog_cache is None:
        _prog_cache = build_program()
    return _prog_cache


def kernel(**inputs):
    proj = np.asarray(inputs["projected"], dtype=np.float32)
    bnds = np.asarray(inputs["boundaries"])
    slot = np.asarray(inputs["slot_mask"])
    W1 = np.asarray(inputs["W1"], dtype=np.float32)
    b1 = np.ascontiguousarray(np.asarray(inputs["b1"], dtype=np.float32))
    W2 = np.asarray(inputs["W2"], dtype=np.float32).reshape(HQ)

    live = slot > 0
    starts = np.where(live, bnds[..., 0], 0).astype(np.int16)     # [B, K]
    ends = np.where(live, bnds[..., 1], 0).astype(np.int16)

    # h-major fp8 for scores: [B, p, half, T]
    projq = np.ascontiguousarray(
        proj.astype(ml_dtypes.float8_e4m3)
        .transpose(0, 2, 1)
        .reshape(B, 2, CH, T)
        .transpose(0, 2, 1, 3)
    )
    # [B, T, H+1] (ones col baked in) -> [B, G, p, g, h]
    proj_bf = np.empty((B, T, H + 1), dtype=ml_dtypes.bfloat16)
    proj_bf[:, :, :H] = proj
    proj_bf[:, :, H] = 1.0
    proj_bf = np.ascontiguousarray(
        proj_bf.reshape(B, NCH // GRP, GRP, CH, H + 1).transpose(0, 1, 3, 2, 4)
    )

    # W1 packed for DoubleRow: [p, half, hq]
    wpack = np.ascontiguousarray(
        W1.reshape(2, CH, HQ).transpose(1, 0, 2)
    ).astype(ml_dtypes.float8_e4m3)
    w2t = W2.astype(ml_dtypes.bfloat16)

    nc = _get_program()
    in_maps = []
    for i in range(NCORES):
        lo, hi = i * BPC, (i + 1) * BPC
        in_maps.append(
            {
                "proj": proj_bf[lo:hi],
                "projq": projq[lo:hi],
                "bounds": np.ascontiguousarray(
                    np.stack([starts[lo:hi], ends[lo:hi]])
                ),
                "wpack": wpack,
                "w2t": w2t,
                "b1": b1,
            }
        )

    res = run_bass_kernel_spmd(nc, in_maps, core_ids=list(range(NCORES)))
    global LAST_RESULTS
    LAST_RESULTS = res
    raw = np.concatenate(
        [np.asarray(r["out"]) for r in res.results], axis=0
    ).astype(np.float32)                                           # [B, K, H+1]
    num, den = raw[..., :H], raw[..., H:]
    return num / np.where(den > 0, den, 1.0)
